# revision 20
# baseline (speedup 1.0000x reference)
"""Trainium2 Bass kernel for nn_KnowledgeFusion (v4).

Math (b=8, H=W=32, d=o=256, n_obj=15, n=16 with appended mean-emb):
  embs_aug = concat([embs, mean(embs)])                  [b,16,256]
  mask     = rasterized boxes (rounded to PATCH_SIZE=2)  [b,16,1024] in {0,1}
  proj     = patches @ Wp                                [b,1024,256]
  inj      = embs_aug @ We                               [b,16,256]
  s[hw]    = sum_n mask[n,hw]   (>=1: image box row)
  out      = proj + (mask^T @ inj) / s[:,None]           [b,1024,256]

Sharding: data-parallel over batch; core c computes batch c (Wp/We
replicated). Computed transposed, outT[o,hw] = Wp^T @ patchesT +
inj^T @ maskN with maskN = mask/s, one PSUM accumulation group per
[128o x 512hw] tile.

Perf structure (33.8us v1 -> 24.0us v2):
- fp16 streams (fp32 PSUM accumulation), fp16 output; host casts back.
  Input DMA 1.58MB -> 0.79MB, output 1MB -> 0.5MB.
- The HAM clock gate keeps the PE at 1.2GHz until one full free-running
  3.4us activity window is busy (measured: a lone ~4us dummy block does
  NOT reliably flip it).  v4 streams ~6.5us of small dummy matmuls so a
  full window is covered regardless of phase; once flipped the rest of
  the dummies drain at 2x and the real matmuls run at 2.4GHz.
- 3 parallel input streams (sync: loc+weights, scalar: pB, gpsimd: pA),
  output in 4 chunks on sync+scalar only (gpsimd SWDGE out-DMA pays a
  ~2us end-of-kernel drain).
- Short s-chain, interleaved into the dummy stream via program order +
  high_priority: host-packed gridOdd=(i|1)/gridEven=(i&~1) consts make
  the PATCH=2 rounding implicit ((gridOdd>=y0)&(gridEven<=y1)); 1/s is
  the one-op reciprocal_approx_fast on the ones@mask PSUM rows.
"""

import sys

sys.path.insert(0, "/opt/trn_rl_repo")

import numpy as np

import concourse.bass as bass
import concourse.bacc as bacc
import concourse.mybir as mybir
from concourse import tile
from concourse import bass_utils
from concourse.alu_op_type import AluOpType

B, H, W, D = 8, 32, 32, 256
NOBJ, N = 15, 16
HW = H * W
O = 256
FP = mybir.dt.float32
FR = mybir.dt.float32r
F16 = mybir.dt.float16
AF = mybir.ActivationFunctionType

# weights blob (fp16 cols): Wp0 Wp1 We0 We1 eT0 eT1 (15 + 1 spare each)
WB = 2 * O + 2 * O + 2 * N  # 1056
N_WARM_PRE = 22  # dummy matmuls (N=256), sized to end at input-arrival
N_WARM_POST = 0
WE = 2 * O + 2 * N  # wbE cols: We0 We1 eT0 eT1


def _bcast(ap, free_dims):
    """AP with explicit free-dim [step, count] pairs (step 0 = broadcast)."""
    return bass.AP(ap.tensor, ap.offset, ap.ap[:1] + free_dims)


def _rows(ap, n):
    """AP restricted to the first n partitions."""
    return bass.AP(ap.tensor, ap.offset, [[ap.ap[0][0], n]] + ap.ap[1:])


def build_nc(debug: bool = False):
    nc = bacc.Bacc("TRN2", target_bir_lowering=False, debug=debug, num_devices=B)

    # loc: [y0 x0 y1 x1 | gridOdd(32) | gridEven(32)] per mask row, fp32
    loc = nc.dram_tensor("loc", [N, 68], FP, kind="ExternalInput")
    wbE = nc.dram_tensor("wbE", [128, WE], F16, kind="ExternalInput")
    wbP = nc.dram_tensor("wbP", [128, 2 * O], F16, kind="ExternalInput")
    pA = nc.dram_tensor("pA", [128, HW], F16, kind="ExternalInput")
    pB = nc.dram_tensor("pB", [128, HW], F16, kind="ExternalInput")
    outT = nc.dram_tensor("outT", [O, HW], F16, kind="ExternalOutput")

    with tile.TileContext(nc) as tc:
        with (
            nc.allow_low_precision(reason="fp16 streams, fp32 PSUM accumulation"),
            tc.tile_pool(name="big", bufs=1) as big,
            tc.tile_pool(name="small", bufs=1) as small,
            tc.tile_pool(name="outp", bufs=1) as outp,
            tc.tile_pool(name="warmP", bufs=1, space=bass.MemorySpace.PSUM) as warmP,
            tc.tile_pool(name="mainP", bufs=4, space=bass.MemorySpace.PSUM) as mainP,
            tc.tile_pool(name="smallP", bufs=1, space=bass.MemorySpace.PSUM) as smallP,
        ):
            # ---- input DMAs, 3 parallel streams; pA/pB in per-half chunks
            # so each chunk's completion sem fires as soon as possible.
            loc_sb = small.tile([N, 68], FP, name="loc_sb")
            nc.sync.dma_start(loc_sb[:], loc[:])
            wbE_sb = big.tile([128, WE], F16, name="wbE_sb")
            nc.sync.dma_start(wbE_sb[:], wbE[:])
            wbP_sb = big.tile([128, 2 * O], F16, name="wbP_sb")
            nc.sync.dma_start(wbP_sb[:], wbP[:])
            pB_sb = big.tile([128, HW], F16, name="pB_sb")
            nc.scalar.dma_start(pB_sb[:, 0:512], pB[:, 0:512])
            nc.scalar.dma_start(pB_sb[:, 512:HW], pB[:, 512:HW])
            pA_sb = big.tile([128, HW], F16, name="pA_sb")
            nc.gpsimd.dma_start(pA_sb[:, 0:512], pA[:, 0:512])
            nc.gpsimd.dma_start(pA_sb[:, 512:HW], pA[:, 512:HW])

            Wp_sb = [wbP_sb[:, O * k : O * (k + 1)] for k in range(2)]
            We_sb = [wbE_sb[:, O * k : O * (k + 1)] for k in range(2)]
            eT_sb = [wbE_sb[:, 2 * O + N * k : 2 * O + N * k + NOBJ] for k in range(2)]

            # ---- PE warm-up stream: N=256 zero matmuls with no data deps.
            warm_sb = big.tile([128, 256], F16, name="warm_sb")
            nc.vector.memset(warm_sb[:], 0.0)
            ones1c = small.tile([N, 1], FP, name="ones1c")
            nc.vector.memset(ones1c[:], 1.0)
            ones16 = small.tile([N, N], FR, name="ones16")
            nc.vector.tensor_copy(ones16[:], _bcast(ones1c[:], [[0, N]]))
            warm_ps = warmP.tile([128, 256], FP, name="warm_ps")
            for _ in range(N_WARM_PRE):
                nc.tensor.matmul(
                    warm_ps[:], warm_sb[:, 0:128], warm_sb[:], start=True, stop=True
                )

            # ---- interval masks on vector:
            # (gridOdd>=y0)&(gridEven<=y1) == the PATCH=2-rounded box test.
            gridO = loc_sb[:, 4:36]
            gridE = loc_sb[:, 36:68]
            tmpr = small.tile([N, 32], FP, name="tmpr")
            rowm = small.tile([N, 32], FP, name="rowm")
            tmpc = small.tile([N, 32], FP, name="tmpc")
            colm = small.tile([N, 32], FP, name="colm")
            with tc.high_priority():
                nc.vector.tensor_scalar(
                    tmpr[:], gridE, loc_sb[:, 2:3], None, op0=AluOpType.is_le
                )
                nc.vector.scalar_tensor_tensor(
                    rowm[:], gridO, loc_sb[:, 0:1], tmpr[:],
                    op0=AluOpType.is_ge, op1=AluOpType.mult,
                )
                nc.vector.tensor_scalar(
                    tmpc[:], gridE, loc_sb[:, 3:4], None, op0=AluOpType.is_le
                )
                nc.vector.scalar_tensor_tensor(
                    colm[:], gridO, loc_sb[:, 1:2], tmpc[:],
                    op0=AluOpType.is_ge, op1=AluOpType.mult,
                )

            # ---- per 512-pixel half: outer-product mask, s = ones@mask
            # (rows of psumS all equal s), 1/s via single-op approx recip.
            # The appended mean-emb row pairs with the all-ones image-box
            # mask row, so its contribution folds into the 15 real rows:
            #   out_inj = sum_k inj[k] * (mask[k] + 1/15) / s
            # -> maskN = (mask + 1/15) * recS on 15 rows, and no on-device
            # embedding mean at all (inj matmul contracts K=15).
            mask_sb = small.tile([N, HW], FR, name="mask_sb")
            maskN_sb = small.tile([NOBJ, HW], F16, name="maskN_sb")
            recS = small.tile([N, HW], FP, name="recS")
            psumS = [smallP.tile([N, 512], FP, name=f"psS{h}") for h in range(2)]
            for h in range(2):
                sl = slice(512 * h, 512 * (h + 1))
                with tc.high_priority():
                    nc.vector.tensor_tensor(
                        _bcast(mask_sb[:, sl], [[W, 16], [1, W]]),
                        _bcast(rowm[:, 16 * h : 16 * h + 16], [[1, 16], [0, W]]),
                        _bcast(colm[:], [[0, 16], [1, W]]),
                        op=AluOpType.mult,
                    )
            for h in range(2):
                sl = slice(512 * h, 512 * (h + 1))
                nc.tensor.matmul(
                    psumS[h][:], ones16[:], mask_sb[:, sl], start=True, stop=True
                )
                with tc.high_priority():
                    nc.vector.reciprocal_approx_fast(recS[:, sl], psumS[h][:])
                    nc.vector.scalar_tensor_tensor(
                        maskN_sb[:, sl], _rows(mask_sb[:, sl], NOBJ), 1.0 / NOBJ,
                        _rows(recS[:, sl], NOBJ),
                        op0=AluOpType.add, op1=AluOpType.mult,
                    )

            for _ in range(N_WARM_POST):
                nc.tensor.matmul(
                    warm_ps[:], warm_sb[:, 0:128], warm_sb[:], start=True, stop=True
                )

            # ---- inj = embs @ We -> [15, 256] fp16
            psumI = smallP.tile([NOBJ, O], FP, name="psI")
            nc.tensor.matmul(psumI[:], eT_sb[0][:], We_sb[0][:], start=True, stop=False)
            nc.tensor.matmul(psumI[:], eT_sb[1][:], We_sb[1][:], start=False, stop=True)
            inj_sb = small.tile([NOBJ, O], F16, name="inj_sb")
            nc.scalar.activation(inj_sb[:], psumI[:], AF.Copy)

            # ---- main: outT[oc*128:, hc*512:] = Wp^T @ pT + inj^T @ maskN.
            # Whole tiles in sequence (inj last per PSUM group, h0 tiles
            # first) so tile (0,0) completes ASAP and the copy->DMA-out
            # pipeline starts while later tiles still matmul.
            tiles = [(0, 0), (1, 0), (0, 1), (1, 1)]
            cp_eng = [nc.vector, nc.scalar, nc.vector, nc.scalar]
            out_eng = [nc.sync, nc.scalar, nc.sync, nc.scalar]
            for i, (oc, hc) in enumerate(tiles):
                o0, h0 = 128 * oc, 512 * hc
                psum = mainP.tile([128, 512], FP, tag="mps", name=f"ps{oc}{hc}")
                nc.tensor.matmul(
                    psum[:], Wp_sb[0][:, o0 : o0 + 128], pA_sb[:, h0 : h0 + 512],
                    start=True, stop=False,
                )
                nc.tensor.matmul(
                    psum[:], Wp_sb[1][:, o0 : o0 + 128], pB_sb[:, h0 : h0 + 512],
                    start=False, stop=False,
                )
                nc.tensor.matmul(
                    psum[:], inj_sb[:, o0 : o0 + 128], maskN_sb[:, h0 : h0 + 512],
                    start=False, stop=True,
                )
                o_sb = outp.tile([128, 512], F16, name=f"osb{i}")
                if cp_eng[i] is nc.scalar:
                    nc.scalar.activation(o_sb[:], psum[:], AF.Copy)
                else:
                    cp_eng[i].tensor_copy(o_sb[:], psum[:])
                out_eng[i].dma_start(outT[o0 : o0 + 128, h0 : h0 + 512], o_sb[:])

    nc.compile()
    return nc


def make_in_maps(inputs):
    patches = np.asarray(inputs["patches"], dtype=np.float32)
    embs = np.asarray(inputs["embs"], dtype=np.float32)
    locations = np.asarray(inputs["locations"], dtype=np.int32)
    Wp = np.asarray(inputs["Wp"], dtype=np.float16)
    We = np.asarray(inputs["We"], dtype=np.float16)
    img_box = np.array([[0, 0, H, W]], dtype=np.int32)
    gi = np.arange(32)
    grids = np.concatenate([gi | 1, gi & ~1]).astype(np.float32)  # [64]
    grids16 = np.broadcast_to(grids, (N, 64))
    wbE_common = np.zeros((128, WE), dtype=np.float16)
    wbE_common[:, 0:O] = We[0:128]
    wbE_common[:, O : 2 * O] = We[128:256]
    wbP_a = np.ascontiguousarray(
        np.concatenate([Wp[0:128], Wp[128:256]], axis=1)
    )  # [128, 512]
    in_maps = []
    for b in range(B):
        locf = np.concatenate([locations[b], img_box], 0).astype(np.float32)
        eTb = embs[b].T.astype(np.float16)  # [256, 15]
        wbb = wbE_common.copy()
        wbb[:, 2 * O : 2 * O + NOBJ] = eTb[0:128]
        wbb[:, 2 * O + N : 2 * O + N + NOBJ] = eTb[128:256]
        pTb = patches[b].reshape(HW, D).T.astype(np.float16)  # [256, 1024]
        in_maps.append(
            {
                "loc": np.ascontiguousarray(
                    np.concatenate([locf, grids16], axis=1), dtype=np.float32
                ),
                "wbE": wbb,
                "wbP": wbP_a,
                "pA": np.ascontiguousarray(pTb[0:128]),
                "pB": np.ascontiguousarray(pTb[128:256]),
            }
        )
    return in_maps


_NC = None


def _get_nc():
    global _NC
    if _NC is None:
        _NC = build_nc(debug=False)
    return _NC


def run(inputs, trace: bool = False, **kwargs):
    nc = _get_nc()
    res = bass_utils.run_bass_kernel_spmd(
        nc, make_in_maps(inputs), core_ids=list(range(B)), trace=trace, **kwargs
    )
    full = np.stack([res.results[b]["outT"].T for b in range(B)], axis=0)
    return np.ascontiguousarray(full).astype(np.float32), res


def kernel(**inputs) -> np.ndarray:
    full, _ = run(inputs, trace=False)
    return full


# revision 23
# speedup vs baseline: 1.1056x; 1.1056x over previous
"""Trainium2 Bass kernel for nn_KnowledgeFusion (v4).

Math (b=8, H=W=32, d=o=256, n_obj=15, n=16 with appended mean-emb):
  embs_aug = concat([embs, mean(embs)])                  [b,16,256]
  mask     = rasterized boxes (rounded to PATCH_SIZE=2)  [b,16,1024] in {0,1}
  proj     = patches @ Wp                                [b,1024,256]
  inj      = embs_aug @ We                               [b,16,256]
  s[hw]    = sum_n mask[n,hw]   (>=1: image box row)
  out      = proj + (mask^T @ inj) / s[:,None]           [b,1024,256]

Sharding: data-parallel over batch; core c computes batch c (Wp/We
replicated). Computed transposed, outT[o,hw] = Wp^T @ patchesT +
inj^T @ maskN with maskN = mask/s, one PSUM accumulation group per
[128o x 512hw] tile.

Perf structure (33.8us v1 -> 24.0us v2):
- fp16 streams (fp32 PSUM accumulation), fp16 output; host casts back.
  Input DMA 1.58MB -> 0.79MB, output 1MB -> 0.5MB.
- The HAM clock gate keeps the PE at 1.2GHz until one full free-running
  3.4us activity window is busy (measured: a lone ~4us dummy block does
  NOT reliably flip it).  v4 streams ~6.5us of small dummy matmuls so a
  full window is covered regardless of phase; once flipped the rest of
  the dummies drain at 2x and the real matmuls run at 2.4GHz.
- 3 parallel input streams (sync: loc+weights, scalar: pB, gpsimd: pA),
  output in 4 chunks on sync+scalar only (gpsimd SWDGE out-DMA pays a
  ~2us end-of-kernel drain).
- Short s-chain, interleaved into the dummy stream via program order +
  high_priority: host-packed gridOdd=(i|1)/gridEven=(i&~1) consts make
  the PATCH=2 rounding implicit ((gridOdd>=y0)&(gridEven<=y1)); 1/s is
  the one-op reciprocal_approx_fast on the ones@mask PSUM rows.
"""

import sys

sys.path.insert(0, "/opt/trn_rl_repo")

import numpy as np

import concourse.bass as bass
import concourse.bacc as bacc
import concourse.mybir as mybir
from concourse import tile
from concourse import bass_utils
from concourse.alu_op_type import AluOpType

B, H, W, D = 8, 32, 32, 256
NOBJ, N = 15, 16
HW = H * W
O = 256
FP = mybir.dt.float32
FR = mybir.dt.float32r
F16 = mybir.dt.float16
AF = mybir.ActivationFunctionType

# weights blob (fp16 cols): Wp0 Wp1 We0 We1 eT0 eT1 (15 + 1 spare each)
WB = 2 * O + 2 * O + 2 * N  # 1056
N_WARM_PRE = 22  # dummy matmuls (N=256), sized to end at input-arrival
N_WARM_POST = 0
WE = 2 * O + 2 * N  # wbE cols: We0 We1 eT0 eT1


def _bcast(ap, free_dims):
    """AP with explicit free-dim [step, count] pairs (step 0 = broadcast)."""
    return bass.AP(ap.tensor, ap.offset, ap.ap[:1] + free_dims)


def _rows(ap, n):
    """AP restricted to the first n partitions."""
    return bass.AP(ap.tensor, ap.offset, [[ap.ap[0][0], n]] + ap.ap[1:])


def build_nc(debug: bool = False):
    nc = bacc.Bacc("TRN2", target_bir_lowering=False, debug=debug, num_devices=B)

    # loc: [y0 x0 y1 x1 | gridOdd(32) | gridEven(32)] per mask row, fp32
    loc = nc.dram_tensor("loc", [N, 68], FP, kind="ExternalInput")
    wbE = nc.dram_tensor("wbE", [128, WE], F16, kind="ExternalInput")
    wbP = nc.dram_tensor("wbP", [128, 2 * O], F16, kind="ExternalInput")
    pA = nc.dram_tensor("pA", [128, HW], F16, kind="ExternalInput")
    pB = nc.dram_tensor("pB", [128, HW], F16, kind="ExternalInput")
    outT = nc.dram_tensor("outT", [O, HW], F16, kind="ExternalOutput")

    with tile.TileContext(nc) as tc:
        with (
            nc.allow_low_precision(reason="fp16 streams, fp32 PSUM accumulation"),
            tc.tile_pool(name="big", bufs=1) as big,
            tc.tile_pool(name="small", bufs=1) as small,
            tc.tile_pool(name="outp", bufs=1) as outp,
            tc.tile_pool(name="warmP", bufs=1, space=bass.MemorySpace.PSUM) as warmP,
            tc.tile_pool(name="mainP", bufs=4, space=bass.MemorySpace.PSUM) as mainP,
            tc.tile_pool(name="smallP", bufs=1, space=bass.MemorySpace.PSUM) as smallP,
        ):
            # ---- input DMAs. Transfers are FIFO within a ring and all
            # active rings round-robin the same 16 SDMA engines, so the
            # sync ring carries everything except pB in need-order
            # (loc -> We/eT -> Wp -> pA); pB rides the scalar ring in
            # parallel. gpsimd (SWDGE) stays idle entirely.
            loc_sb = small.tile([N, 68], FP, name="loc_sb")
            nc.sync.dma_start(loc_sb[:], loc[:])
            wbE_sb = big.tile([128, WE], F16, name="wbE_sb")
            nc.sync.dma_start(wbE_sb[:], wbE[:])
            pB_sb = big.tile([128, HW], F16, name="pB_sb")
            nc.scalar.dma_start(pB_sb[:], pB[:])
            wbP_sb = big.tile([128, 2 * O], F16, name="wbP_sb")
            nc.sync.dma_start(wbP_sb[:], wbP[:])
            pA_sb = big.tile([128, HW], F16, name="pA_sb")
            nc.sync.dma_start(pA_sb[:], pA[:])

            Wp_sb = [wbP_sb[:, O * k : O * (k + 1)] for k in range(2)]
            We_sb = [wbE_sb[:, O * k : O * (k + 1)] for k in range(2)]
            eT_sb = [wbE_sb[:, 2 * O + N * k : 2 * O + N * k + NOBJ] for k in range(2)]

            # ---- PE warm-up stream: N=256 zero matmuls with no data deps.
            warm_sb = big.tile([128, 256], F16, name="warm_sb")
            nc.vector.memset(warm_sb[:], 0.0)
            ones1c = small.tile([N, 1], FP, name="ones1c")
            nc.vector.memset(ones1c[:], 1.0)
            ones16 = small.tile([N, N], FR, name="ones16")
            nc.vector.tensor_copy(ones16[:], _bcast(ones1c[:], [[0, N]]))
            warm_ps = warmP.tile([128, 256], FP, name="warm_ps")
            for _ in range(N_WARM_PRE):
                nc.tensor.matmul(
                    warm_ps[:], warm_sb[:, 0:128], warm_sb[:], start=True, stop=True
                )

            # ---- interval masks on vector:
            # (gridOdd>=y0)&(gridEven<=y1) == the PATCH=2-rounded box test.
            gridO = loc_sb[:, 4:36]
            gridE = loc_sb[:, 36:68]
            tmpr = small.tile([N, 32], FP, name="tmpr")
            rowm = small.tile([N, 32], FP, name="rowm")
            tmpc = small.tile([N, 32], FP, name="tmpc")
            colm = small.tile([N, 32], FP, name="colm")
            with tc.high_priority():
                nc.vector.tensor_scalar(
                    tmpr[:], gridE, loc_sb[:, 2:3], None, op0=AluOpType.is_le
                )
                nc.vector.scalar_tensor_tensor(
                    rowm[:], gridO, loc_sb[:, 0:1], tmpr[:],
                    op0=AluOpType.is_ge, op1=AluOpType.mult,
                )
                nc.vector.tensor_scalar(
                    tmpc[:], gridE, loc_sb[:, 3:4], None, op0=AluOpType.is_le
                )
                nc.vector.scalar_tensor_tensor(
                    colm[:], gridO, loc_sb[:, 1:2], tmpc[:],
                    op0=AluOpType.is_ge, op1=AluOpType.mult,
                )

            # ---- per 512-pixel half: outer-product mask, s = ones@mask
            # (rows of psumS all equal s), 1/s via single-op approx recip.
            # The appended mean-emb row pairs with the all-ones image-box
            # mask row, so its contribution folds into the 15 real rows:
            #   out_inj = sum_k inj[k] * (mask[k] + 1/15) / s
            # -> maskN = (mask + 1/15) * recS on 15 rows, and no on-device
            # embedding mean at all (inj matmul contracts K=15).
            mask_sb = small.tile([N, HW], FR, name="mask_sb")
            maskN_sb = small.tile([NOBJ, HW], F16, name="maskN_sb")
            recS = small.tile([N, HW], FP, name="recS")
            psumS = [smallP.tile([N, 512], FP, name=f"psS{h}") for h in range(2)]
            for h in range(2):
                sl = slice(512 * h, 512 * (h + 1))
                with tc.high_priority():
                    nc.vector.tensor_tensor(
                        _bcast(mask_sb[:, sl], [[W, 16], [1, W]]),
                        _bcast(rowm[:, 16 * h : 16 * h + 16], [[1, 16], [0, W]]),
                        _bcast(colm[:], [[0, 16], [1, W]]),
                        op=AluOpType.mult,
                    )
            for h in range(2):
                sl = slice(512 * h, 512 * (h + 1))
                nc.tensor.matmul(
                    psumS[h][:], ones16[:], mask_sb[:, sl], start=True, stop=True
                )
            for h in range(2):
                sl = slice(512 * h, 512 * (h + 1))
                with tc.high_priority():
                    nc.vector.reciprocal_approx_fast(recS[:, sl], psumS[h][:])
                    nc.vector.scalar_tensor_tensor(
                        maskN_sb[:, sl], _rows(mask_sb[:, sl], NOBJ), 1.0 / NOBJ,
                        _rows(recS[:, sl], NOBJ),
                        op0=AluOpType.add, op1=AluOpType.mult,
                    )

            for _ in range(N_WARM_POST):
                nc.tensor.matmul(
                    warm_ps[:], warm_sb[:, 0:128], warm_sb[:], start=True, stop=True
                )

            # ---- inj = embs @ We -> [15, 256] fp16
            psumI = smallP.tile([NOBJ, O], FP, name="psI")
            nc.tensor.matmul(psumI[:], eT_sb[0][:], We_sb[0][:], start=True, stop=False)
            nc.tensor.matmul(psumI[:], eT_sb[1][:], We_sb[1][:], start=False, stop=True)
            inj_sb = small.tile([NOBJ, O], F16, name="inj_sb")
            nc.scalar.activation(inj_sb[:], psumI[:], AF.Copy)

            # ---- main: outT[oc*128:, hc*512:] = Wp^T @ pT + inj^T @ maskN.
            # Whole tiles in sequence (inj last per PSUM group, h0 tiles
            # first) so tile (0,0) completes ASAP and the copy->DMA-out
            # pipeline starts while later tiles still matmul.
            # phase-ordered to match operand arrival: all WpB (pB lands
            # first), then all WpA (pA is last on the sync ring), then the
            # inj stops — h0 tiles first so copy+DMA-out start early.
            tiles = [(0, 0), (1, 0), (0, 1), (1, 1)]
            psums = {}
            for oc, hc in tiles:
                psums[(oc, hc)] = mainP.tile(
                    [128, 512], FP, tag="mps", name=f"ps{oc}{hc}"
                )
                nc.tensor.matmul(
                    psums[(oc, hc)][:], Wp_sb[1][:, 128 * oc : 128 * oc + 128],
                    pB_sb[:, 512 * hc : 512 * hc + 512],
                    start=True, stop=False,
                )
            for oc, hc in tiles:
                nc.tensor.matmul(
                    psums[(oc, hc)][:], Wp_sb[0][:, 128 * oc : 128 * oc + 128],
                    pA_sb[:, 512 * hc : 512 * hc + 512],
                    start=False, stop=False,
                )
            cp_eng = [nc.vector, nc.scalar, nc.vector, nc.scalar]
            out_eng = [nc.sync, nc.scalar, nc.sync, nc.scalar]
            for i, (oc, hc) in enumerate(tiles):
                o0, h0 = 128 * oc, 512 * hc
                nc.tensor.matmul(
                    psums[(oc, hc)][:], inj_sb[:, o0 : o0 + 128],
                    maskN_sb[:, h0 : h0 + 512],
                    start=False, stop=True,
                )
                o_sb = outp.tile([128, 512], F16, name=f"osb{i}")
                if cp_eng[i] is nc.scalar:
                    nc.scalar.activation(o_sb[:], psums[(oc, hc)][:], AF.Copy)
                else:
                    cp_eng[i].tensor_copy(o_sb[:], psums[(oc, hc)][:])
                out_eng[i].dma_start(outT[o0 : o0 + 128, h0 : h0 + 512], o_sb[:])

    nc.compile()
    return nc


def make_in_maps(inputs):
    patches = np.asarray(inputs["patches"], dtype=np.float32)
    embs = np.asarray(inputs["embs"], dtype=np.float32)
    locations = np.asarray(inputs["locations"], dtype=np.int32)
    Wp = np.asarray(inputs["Wp"], dtype=np.float16)
    We = np.asarray(inputs["We"], dtype=np.float16)
    img_box = np.array([[0, 0, H, W]], dtype=np.int32)
    gi = np.arange(32)
    grids = np.concatenate([gi | 1, gi & ~1]).astype(np.float32)  # [64]
    grids16 = np.broadcast_to(grids, (N, 64))
    wbE_common = np.zeros((128, WE), dtype=np.float16)
    wbE_common[:, 0:O] = We[0:128]
    wbE_common[:, O : 2 * O] = We[128:256]
    wbP_a = np.ascontiguousarray(
        np.concatenate([Wp[0:128], Wp[128:256]], axis=1)
    )  # [128, 512]
    in_maps = []
    for b in range(B):
        locf = np.concatenate([locations[b], img_box], 0).astype(np.float32)
        eTb = embs[b].T.astype(np.float16)  # [256, 15]
        wbb = wbE_common.copy()
        wbb[:, 2 * O : 2 * O + NOBJ] = eTb[0:128]
        wbb[:, 2 * O + N : 2 * O + N + NOBJ] = eTb[128:256]
        pTb = patches[b].reshape(HW, D).T.astype(np.float16)  # [256, 1024]
        in_maps.append(
            {
                "loc": np.ascontiguousarray(
                    np.concatenate([locf, grids16], axis=1), dtype=np.float32
                ),
                "wbE": wbb,
                "wbP": wbP_a,
                "pA": np.ascontiguousarray(pTb[0:128]),
                "pB": np.ascontiguousarray(pTb[128:256]),
            }
        )
    return in_maps


_NC = None


def _get_nc():
    global _NC
    if _NC is None:
        _NC = build_nc(debug=False)
    return _NC


def run(inputs, trace: bool = False, **kwargs):
    nc = _get_nc()
    res = bass_utils.run_bass_kernel_spmd(
        nc, make_in_maps(inputs), core_ids=list(range(B)), trace=trace, **kwargs
    )
    full = np.stack([res.results[b]["outT"].T for b in range(B)], axis=0)
    return np.ascontiguousarray(full).astype(np.float32), res


def kernel(**inputs) -> np.ndarray:
    full, _ = run(inputs, trace=False)
    return full
